# revision 8
# baseline (speedup 1.0000x reference)
"""Trainium2 Bass kernel for a single attention head.

Problem: X[4,4096,1024], Wq/Wk/Wv[1024,128] ->
  softmax((X@Wq)(X@Wk)^T / sqrt(1024)) @ (X@Wv)   -> [4,4096,128]

Sharding: 8 cores = 4 batches x 2 query-halves. Each core receives the full
X of its batch (rolled so its query half is rows [0:2048)), computes K/V for
all 4096 keys and flash-style attention for its 2048 queries.

Pipeline (all matmuls bf16 inputs, fp32 PSUM accumulation):
  - X^T is pre-laid-out and rounded to bf16 on the host (pure relayout),
    so the device does plain chunked DMA loads of X^T. Weights are
    host-prepped to bf16 tiles the same way.
  - Startup: ~24 dummy warmup matmuls (ones x ones into the out PSUM bank,
    overwritten later) run during the initial DMA latency so the PE HAM
    clock-gate is warm (2.4GHz) when real matmuls start; weights are
    DMA'd first and chunks 0-1 arrive in 2-d-tile slices so the first
    projection matmul can start as soon as ~160KB have landed; all
    remaining chunk DMAs are issued up front (the DGE queues drain them
    in order, so data always arrives ahead of the projection consuming
    it).
  - Projections K^T/V^T/Q^T per 512-token chunk with two PSUM banks
    interleaved (K/V pairs); production of chunks 1-7 is interleaved
    into the first attention q-chunk, 4 matmuls per k-step (the qi0
    k-loop is PE-saturated at ~8 matmuls / 1.72us per k-tile).
  - Transposed flash attention, software-pipelined: S^T(kt+1) is issued
    to the PE before O^T(kt) so the PE has work during exp(kt) on ACT.
    The qi1 k-loop is ACT-paced (~1.0us per [128,1024] Exp), so ACT is
    kept free of everything except exp.
  - exp outputs land in a 16-slice ring tile; the softmax denominator
    is accumulated by one contiguous 4-slice [128,4096] DVE add per 4
    k-tiles; the 4 interleaved partials are then tree-reduced by two
    more DVE adds, and only the final cross-partition sum uses a
    ones-matmul on the PE (1 matmul per 512 queries instead of 8).
  - Deferred epilogue: out_ps is evacuated by a DVE copy right after
    the last O matmul; the l -> 1/l -> scale chain for q-chunk 0 runs
    inside q-chunk 1's loop. The final q-chunk skips the evacuation
    (DVE multiplies straight out of PSUM) and runs a quartered
    mul->DMA pipeline; its post-exp(31) dependency chain is one
    O-half + one ones-matmul per half.
  - O^T is DMA'd out transposed and un-transposed on the host.
"""

import numpy as np

B, N, D, H = 4, 4096, 1024, 128
NCORES = 8
QSPLIT = 2  # cores per batch (query halves)
NQ = N // QSPLIT
SCALE = 1.0 / float(np.sqrt(np.float32(D)))
P = 128  # partitions
FB = 512  # matmul free-dim block (one fp32 PSUM bank)
CR = 512  # X rows per projection job
QC = 1024  # query chunk
DT = D // P   # 8 contraction tiles
NT = N // P   # 32 key tiles
NC = N // CR  # 8 projection jobs
XC = 8        # X DMA chunks
XCR = N // XC
KPC = CR // P  # 4 key tiles per chunk
PR = 16       # pT ring depth (slices)
GL = 4        # denominator group length (ring slices per DVE add)
WARM_MM = 40  # HAM warmup dummy matmuls


def emit_attention(tc, XT, Ws, OT, n=N, d=D, nq=NQ):
    """Emit the single-core attention program into TileContext tc."""
    import concourse.mybir as mybir

    nc = tc.nc
    dt = mybir.dt
    f32, bf16 = dt.float32, dt.bfloat16
    AF = mybir.ActivationFunctionType
    qc = QC
    NQC = nq // qc

    from contextlib import ExitStack

    with ExitStack() as ctx:
        cpool = ctx.enter_context(tc.tile_pool(name="const", bufs=1))
        big = ctx.enter_context(tc.tile_pool(name="big", bufs=1))
        vtp = ctx.enter_context(tc.tile_pool(name="vtp", bufs=2))
        gsp = ctx.enter_context(tc.tile_pool(name="gsp", bufs=2))
        epp = ctx.enter_context(tc.tile_pool(name="ep", bufs=2))
        # PSUM: p12 2x1 + stp 2x2 + accp 1x2 = 8 banks
        p12 = ctx.enter_context(tc.tile_pool(name="p12", bufs=2, space="PSUM"))
        stp = ctx.enter_context(tc.tile_pool(name="stps", bufs=2, space="PSUM"))
        accp = ctx.enter_context(tc.tile_pool(name="accps", bufs=1, space="PSUM"))

        ones_sq = cpool.tile([P, P], bf16)
        nc.vector.memset(ones_sq[:], 1.0)

        # ---- HAM warmup: dummy matmuls into the out-accumulator PSUM bank.
        # They only depend on the memset, so they run during the ~9us DMA
        # startup latency; the first real O matmul has start=True and
        # overwrites. Keeps the PE clock-gate at 2.4GHz for the real work.
        warm_ps = accp.tile([P, qc], f32, tag="out", name="warm")
        for _ in range(WARM_MM):
            nc.tensor.matmul(warm_ps[:, 0:P], ones_sq[:], ones_sq[:],
                             start=True, stop=True)

        w_sb = {}

        def load_w(name):
            t = cpool.tile([P, DT * H], bf16, tag=name, name=f"w_{name}")
            nc.sync.dma_start(
                t[:].rearrange("p (t h) -> p t h", t=DT), Ws[name])
            w_sb[name] = t

        # X^T: xt[p, c, t, nb] = X^T[t*128+p, c*1024+nb] (DMA-chunk major)
        xt = big.tile([P, XC * DT * XCR], bf16)
        xt4 = xt[:].rearrange("p (c t nb) -> p c t nb", c=XC, t=DT)

        def xt_job(hc, t):
            """[128, 512] X^T slice for projection job hc, d-tile t."""
            c = hc * CR // XCR
            o = (hc * CR) % XCR
            return xt4[:, c, t, o:o + CR]
        kT = big.tile([P, n], bf16)          # K^T[h, keys]
        qT = big.tile([P, nq], bf16)         # Q^T[h, q]
        v_sb = big.tile([P, NT * H], bf16)   # V[k%128, kt*H + h]
        v_sb3 = v_sb[:].rearrange("p (kt h) -> p kt h", h=H)
        # exp ring: pT3[:, r, :] = P^T slice for k-tile with kt % PR == r
        pT_all = big.tile([P, PR * qc], bf16)
        pT3 = pT_all[:].rearrange("p (r q) -> p r q", r=PR)

        def produce_data(c):
            nc.sync.dma_start(xt4[:, c], XT[c])

        def produce_slice(c, h2):
            """4-d-tile (512KB) half of chunk c: fine-grained arrival
            while keeping 4KB per-partition DMA lines (2KB lines halve
            the early DMA bandwidth)."""
            nc.sync.dma_start(xt4[:, c, 4 * h2:4 * h2 + 4],
                              XT[c][:, 4 * h2:4 * h2 + 4])

        def proj_pair_stages(jobs, on_scalar=False):
            """Return 4 closures, each emitting 2 t-steps of the pair's
            interleaved matmuls; the last also emits copies/transposes."""
            state = {}

            def stage(si):
                def run():
                    if si == 0:
                        state['tiles'] = [
                            p12.tile([P, CR], f32, tag="pps",
                                     name=f"ps_{w}{c}")
                            for w, c in jobs]
                    for t in range(si * 2, si * 2 + 2):
                        for (wname, c), ps in zip(jobs, state['tiles']):
                            nc.tensor.matmul(
                                ps[:],
                                w_sb[wname][:, t * H:(t + 1) * H],
                                xt_job(c, t),
                                start=(t == 0),
                                stop=(t == DT - 1),
                            )
                    if si == 3:
                        for (wname, c), ps in zip(jobs, state['tiles']):
                            cp = (nc.scalar.copy if on_scalar
                                  else nc.vector.tensor_copy)
                            if wname == "wv":
                                vt = vtp.tile([P, CR], bf16, tag="vt",
                                              name=f"vt{c}")
                                cp(vt[:], ps[:])
                                nc.sync.dma_start_transpose(
                                    v_sb3[:, c * KPC:(c + 1) * KPC], vt[:])
                            else:
                                dst = kT if wname == "wk" else qT
                                cp(dst[:, c * CR:(c + 1) * CR], ps[:])
                return run
            return [stage(i) for i in range(4)]

        def proj_pair(jobs, on_scalar=False):
            for s in proj_pair_stages(jobs, on_scalar):
                s()

        # ---- Phase 1: weights first, then X chunks fine-grained/early.
        # All DMA triggers are issued up front in need-order; the DGE
        # queues execute them FIFO so later chunks never delay earlier
        # ones, and the first projection matmul only waits for
        # wk + wv + one 2-t-slice of chunk 0 (~0.75MB).
        load_w("wk")
        load_w("wv")
        produce_slice(0, 0)
        produce_slice(0, 1)
        load_w("wq")
        produce_data(1)
        for c in range(2, XC):
            produce_data(c)
        proj_pair((("wk", 0), ("wv", 0)), on_scalar=True)
        proj_pair((("wq", 0), ("wq", 1)), on_scalar=True)

        def emit_S(q0, kt):
            st = stp.tile([P, qc], f32, tag="st", name=f"st{q0}_{kt}")
            for j in range(0, qc, FB):
                nc.tensor.matmul(
                    st[:, j:j + FB],
                    kT[:, kt * P:(kt + 1) * P],
                    qT[:, q0 + j:q0 + j + FB],
                    start=True, stop=True,
                )
            return st

        # deferred epilogue state from the previous q-chunk
        pending = {}

        def defer_tree_a():
            # acc2 = acc4[0]+acc4[1] | acc4[2]+acc4[3]  (interleaved pairs)
            if not pending:
                return
            a4 = pending['acc4']
            acc2 = epp.tile([P, 2 * qc], bf16, tag="acc2", bufs=1,
                            name="acc2")
            nc.vector.tensor_add(acc2[:], a4[:, 0:2 * qc], a4[:, 2 * qc:])
            pending['acc2'] = acc2

        def defer_tree_b():
            if not pending:
                return
            acc2 = pending.pop('acc2')
            accf = epp.tile([P, qc], bf16, tag="accf", bufs=1, name="accf")
            nc.vector.tensor_add(accf[:], acc2[:, 0:qc], acc2[:, qc:])
            pending['accf'] = accf

        def finish_epilogue():
            if not pending:
                return
            accf, ob, q0p = pending.pop('accf'), pending.pop('ob'), \
                pending.pop('q0')
            pending.pop('acc4')
            l_a = p12.tile([P, FB], f32, tag="pps", name=f"la{q0p}")
            l_b = p12.tile([P, FB], f32, tag="pps", name=f"lb{q0p}")
            r_sb = epp.tile([P, qc], f32, tag="rsb", name=f"rsb{q0p}")
            o_sb = epp.tile([P, qc], f32, tag="osb", name=f"osb{q0p}")
            nc.tensor.matmul(l_a[:], ones_sq[:], accf[:, 0:FB],
                             start=True, stop=True)
            nc.vector.reciprocal_approx_fast(r_sb[:, 0:FB], l_a[:])
            nc.tensor.matmul(l_b[:], ones_sq[:], accf[:, FB:qc],
                             start=True, stop=True)
            nc.gpsimd.tensor_mul(o_sb[:, 0:FB], ob[:, 0:FB], r_sb[:, 0:FB])
            nc.sync.dma_start(OT[:, q0p:q0p + FB], o_sb[:, 0:FB])
            nc.vector.reciprocal_approx_fast(r_sb[:, FB:qc], l_b[:])
            nc.vector.tensor_mul(o_sb[:, FB:qc], ob[:, FB:qc], r_sb[:, FB:qc])
            nc.sync.dma_start(OT[:, q0p + FB:q0p + qc], o_sb[:, FB:qc])

        for qi in range(NQC):
            q0 = qi * qc
            final = (qi == NQC - 1)
            actions = {}
            if qi == 0:
                pjobs = [(("wk", c), ("wv", c)) for c in range(1, NC)]
                pjobs.append((("wq", 2), ("wq", 3)))
                # K1/V1 compressed into the first two slots (needed by S(4))
                # All in-loop proj copies run on ACT (exp leaves ~700ns/kt
                # of ACT slack in qi0) so the DVE never gates the p12
                # PSUM rotation from behind a 2.3us denominator add.
                s10, s11, s12, s13 = proj_pair_stages(pjobs[0],
                                                      on_scalar=True)
                actions.setdefault(0, []).extend([(s10, ()), (s11, ())])
                actions.setdefault(1, []).extend([(s12, ()), (s13, ())])
                at = 2
                for jobs in pjobs[1:]:
                    for s in proj_pair_stages(jobs, on_scalar=True):
                        actions.setdefault(at, []).append((s, ()))
                        at += 1
            else:
                actions.setdefault(1, []).append((defer_tree_a, ()))
                actions.setdefault(2, []).append((defer_tree_b, ()))
                actions.setdefault(5, []).append((finish_epilogue, ()))

            out_ps = accp.tile([P, qc], f32, tag="out", name=f"out{qi}")
            st_tiles = {0: emit_S(q0, 0)}
            # denominator accumulator: [p, 4, qc] bf16, four interleaved
            # partial sums tree-reduced on DVE before the epilogue
            # ones-matmul (partition reduction only).
            acc4 = gsp.tile([P, GL * qc], bf16, tag="a4", name=f"a4_{qi}")
            fin = {}  # final-chunk epilogue tiles
            for kt in range(NT):
                # S(kt+1) is emitted FIRST so it can never queue behind a
                # stalled projection matmul in the PE FIFO (the proj
                # pipeline is gated on copies; S leading breaks the
                # proj->copy->exp->S convoy cycle).
                if kt + 1 < NT:
                    st_tiles[kt + 1] = emit_S(q0, kt + 1)
                # exp on ACT into the ring
                nc.scalar.activation(
                    pT3[:, kt % PR, :], st_tiles.pop(kt)[:],
                    AF.Exp, scale=SCALE)
                # O^T accumulation for the PREVIOUS kt (software pipeline)
                if kt > 0:
                    for j in range(0, qc, FB):
                        nc.tensor.matmul(
                            out_ps[:, j:j + FB],
                            v_sb3[:, kt - 1, :],
                            pT3[:, (kt - 1) % PR, j:j + FB],
                            start=(kt - 1 == 0), stop=False,
                        )
                for fn, arg in actions.get(kt, ()):
                    fn(*arg)
                # denominator: one contiguous 4-slice DVE add per 4 k-tiles.
                # The final q-chunk keeps its last 4 k-tiles out of the
                # accumulator so the post-exp(31) chain stays short; its
                # tree reduction runs inside the loop (kt 29-31).
                ngrp = NT - GL if final else NT
                if kt < ngrp and kt % GL == GL - 1:
                    r0 = (kt - (GL - 1)) % PR
                    grp = pT_all[:, r0 * qc:(r0 + GL) * qc]
                    if kt == GL - 1:
                        nc.vector.tensor_copy(acc4[:], grp)
                    else:
                        nc.vector.tensor_add(acc4[:], acc4[:], grp)
                if final:
                    if kt == 29:
                        a2 = epp.tile([P, 2 * qc], bf16, tag="acc2f",
                                      bufs=1, name="acc2f")
                        nc.vector.tensor_add(
                            a2[:], acc4[:, 0:2 * qc], acc4[:, 2 * qc:])
                        fin['a2'] = a2
                    elif kt == 30:
                        af = epp.tile([P, qc], bf16, tag="accff", bufs=1,
                                      name="accff")
                        nc.vector.tensor_add(
                            af[:], fin['a2'][:, 0:qc], fin['a2'][:, qc:])
                        fin['af'] = af
                    elif kt == 31:
                        g89 = epp.tile([P, qc], bf16, tag="g89", bufs=1,
                                       name="g89")
                        nc.vector.tensor_add(
                            g89[:], pT3[:, (NT - 4) % PR, :],
                            pT3[:, (NT - 3) % PR, :])
                        fin['g89'] = g89

            if not final:
                # last O^T tile, evacuate on DVE (ACT stays exp-only),
                # defer the l/recip/scale chain into the next q-chunk.
                for j in range(0, qc, FB):
                    nc.tensor.matmul(
                        out_ps[:, j:j + FB],
                        v_sb3[:, NT - 1, :],
                        pT3[:, (NT - 1) % PR, j:j + FB],
                        start=False, stop=True,
                    )
                ob = epp.tile([P, qc], f32, tag="ob", name=f"ob{qi}")
                nc.vector.tensor_copy(ob[:], out_ps[:])
                pending.update(acc4=acc4, ob=ob, q0=q0)
            else:
                # ---- final-chunk tail: everything that can run before
                # exp(31) is emitted first; the post-exp(31) chain is
                # [O31-half, l-last-matmul] x2 -> recip -> mul -> DMA,
                # quartered so scale and DMA-out overlap.
                l_a = p12.tile([P, FB], f32, tag="pps", name="la_f")
                l_b = p12.tile([P, FB], f32, tag="pps", name="lb_f")
                terms = [fin['af'], fin['g89'], pT3[:, (NT - 2) % PR, :]]
                for g, t in enumerate(terms):
                    nc.tensor.matmul(l_a[:], ones_sq[:], t[:, 0:FB],
                                     start=(g == 0), stop=False)
                for g, t in enumerate(terms):
                    nc.tensor.matmul(l_b[:], ones_sq[:], t[:, FB:qc],
                                     start=(g == 0), stop=False)
                last = pT3[:, (NT - 1) % PR, :]
                r_sb = epp.tile([P, qc], f32, tag="rsb", name="rsb_f")
                o_sb = epp.tile([P, qc], f32, tag="osb", name="osb_f")
                for j, l in ((0, l_a), (FB, l_b)):
                    nc.tensor.matmul(
                        out_ps[:, j:j + FB],
                        v_sb3[:, NT - 1, :],
                        pT3[:, (NT - 1) % PR, j:j + FB],
                        start=False, stop=True,
                    )
                    nc.tensor.matmul(l[:], ones_sq[:], last[:, j:j + FB],
                                     start=False, stop=True)
                    nc.vector.reciprocal_approx_fast(r_sb[:, j:j + FB], l[:])
                    for jq in (j, j + FB // 2):
                        nc.vector.tensor_mul(
                            o_sb[:, jq:jq + FB // 2],
                            out_ps[:, jq:jq + FB // 2],
                            r_sb[:, jq:jq + FB // 2])
                        nc.sync.dma_start(OT[:, q0 + jq:q0 + jq + FB // 2],
                                          o_sb[:, jq:jq + FB // 2])

        finish_epilogue()


def build_bass(n=N, d=D, nq=NQ):
    import concourse.mybir as mybir
    from concourse import bacc
    from concourse.tile import TileContext

    dt = mybir.dt
    nc = bacc.Bacc("TRN2", target_bir_lowering=False, debug=False)
    XT = nc.dram_tensor(
        "XT", [XC, P, DT, XCR], dt.bfloat16, kind="ExternalInput").ap()
    Ws = {}
    for name in ("wq", "wk", "wv"):
        Ws[name] = nc.dram_tensor(
            name.upper(), [P, DT, H], dt.bfloat16, kind="ExternalInput").ap()
    OT = nc.dram_tensor("OT", [H, nq], dt.float32, kind="ExternalOutput").ap()

    with TileContext(nc) as tc:
        emit_attention(tc, XT, Ws, OT, n=n, d=d, nq=nq)
    nc.compile()  # bacc passes: split multi-waits into EVSEM chains, etc.
    return nc


_CACHED = {}


def _get_nc():
    if "nc" not in _CACHED:
        _CACHED["nc"] = build_bass()
    return _CACHED["nc"]


def _prep_w(w):
    import ml_dtypes
    # [D, H] f32 -> [128, DT, H] bf16 with w_t[p, t, h] = W[t*128+p, h]
    return np.ascontiguousarray(
        w.reshape(DT, P, H).transpose(1, 0, 2)).astype(ml_dtypes.bfloat16)


def _prep_xt(xb):
    import ml_dtypes
    # [N, D] f32 -> [XC, 128, DT, XCR] bf16:
    # XT[c, p, t, nb] = X[c*XCR+nb, t*128+p]
    x4 = xb.reshape(XC, XCR, DT, P)          # [c, nb, t, p]
    return np.ascontiguousarray(
        x4.transpose(0, 3, 2, 1)).astype(ml_dtypes.bfloat16)


def kernel(X, Wq, Wk, Wv, trace=False):
    """Full-input entry point: X [4,4096,1024] f32 -> [4,4096,128] f32."""
    from concourse.bass_utils import run_bass_kernel_spmd

    X = np.ascontiguousarray(X, dtype=np.float32)
    wmap = {"WQ": _prep_w(np.asarray(Wq, dtype=np.float32)),
            "WK": _prep_w(np.asarray(Wk, dtype=np.float32)),
            "WV": _prep_w(np.asarray(Wv, dtype=np.float32))}

    nc = _get_nc()
    in_maps = []
    for core in range(NCORES):
        b, half = core // QSPLIT, core % QSPLIT
        xb = X[b]
        if half:
            # roll so this core's queries are rows [0:NQ); key set is unchanged
            xb = np.concatenate([xb[NQ:], xb[:NQ]], axis=0)
        in_maps.append({"XT": _prep_xt(xb), **wmap})

    res = run_bass_kernel_spmd(
        nc, in_maps, core_ids=list(range(NCORES)), trace=trace
    )
    out = np.empty((B, N, H), dtype=np.float32)
    for core in range(NCORES):
        b, half = core // QSPLIT, core % QSPLIT
        out[b, half * NQ:(half + 1) * NQ] = res.results[core]["OT"].T
    if trace:
        return out, res
    return out


# revision 15
# speedup vs baseline: 1.0345x; 1.0345x over previous
"""Trainium2 Bass kernel for a single attention head.

Problem: X[4,4096,1024], Wq/Wk/Wv[1024,128] ->
  softmax((X@Wq)(X@Wk)^T / sqrt(1024)) @ (X@Wv)   -> [4,4096,128]

Sharding: 8 cores = 4 batches x 2 query-halves. Each core receives the full
X of its batch (rolled so its query half is rows [0:2048)), computes K/V for
all 4096 keys and flash-style attention for its 2048 queries.

Pipeline (all matmuls bf16 inputs, fp32 PSUM accumulation):
  - X^T is pre-laid-out and rounded to bf16 on the host (pure relayout),
    so the device does plain chunked DMA loads of X^T. Weights are
    host-prepped to bf16 tiles the same way.
  - Startup: ~24 dummy warmup matmuls (ones x ones into the out PSUM bank,
    overwritten later) run during the initial DMA latency so the PE HAM
    clock-gate is warm (2.4GHz) when real matmuls start; weights are
    DMA'd first and chunks 0-1 arrive in 2-d-tile slices so the first
    projection matmul can start as soon as ~160KB have landed; all
    remaining chunk DMAs are issued up front (the DGE queues drain them
    in order, so data always arrives ahead of the projection consuming
    it).
  - Projections K^T/V^T/Q^T per 512-token chunk with two PSUM banks
    interleaved (K/V pairs); production of chunks 1-7 is interleaved
    into the first attention q-chunk, 4 matmuls per k-step (the qi0
    k-loop is PE-saturated at ~8 matmuls / 1.72us per k-tile).
  - Transposed flash attention, software-pipelined: S^T(kt+1) is issued
    to the PE before O^T(kt) so the PE has work during exp(kt) on ACT.
    The qi1 k-loop is ACT-paced (~1.0us per [128,1024] Exp), so ACT is
    kept free of everything except exp.
  - exp outputs land in a 16-slice ring tile; the softmax denominator
    is accumulated by one contiguous 4-slice [128,4096] DVE add per 4
    k-tiles; the 4 interleaved partials are then tree-reduced by two
    more DVE adds, and only the final cross-partition sum uses a
    ones-matmul on the PE (1 matmul per 512 queries instead of 8).
  - Deferred epilogue: out_ps is evacuated by a DVE copy right after
    the last O matmul; the l -> 1/l -> scale chain for q-chunk 0 runs
    inside q-chunk 1's loop. The final q-chunk skips the evacuation
    (DVE multiplies straight out of PSUM) and runs a quartered
    mul->DMA pipeline; its post-exp(31) dependency chain is one
    O-half + one ones-matmul per half.
  - O^T is DMA'd out transposed and un-transposed on the host.
"""

import numpy as np

B, N, D, H = 4, 4096, 1024, 128
NCORES = 8
QSPLIT = 2  # cores per batch (query halves)
NQ = N // QSPLIT
SCALE = 1.0 / float(np.sqrt(np.float32(D)))
P = 128  # partitions
FB = 512  # matmul free-dim block (one fp32 PSUM bank)
CR = 512  # X rows per projection job
QC = 1024  # query chunk
DT = D // P   # 8 contraction tiles
NT = N // P   # 32 key tiles
NC = N // CR  # 8 projection jobs
XC = 8        # X DMA chunks
XCR = N // XC
KPC = CR // P  # 4 key tiles per chunk
PR = 16       # pT ring depth (slices)
GL = 4        # denominator group length (ring slices per DVE add)
WARM_MM = 12  # HAM warmup dummy matmuls (N=512: self-equalizing span)


def emit_attention(tc, XT, Ws, OT, n=N, d=D, nq=NQ):
    """Emit the single-core attention program into TileContext tc."""
    import concourse.mybir as mybir

    nc = tc.nc
    dt = mybir.dt
    f32, bf16 = dt.float32, dt.bfloat16
    AF = mybir.ActivationFunctionType
    qc = QC
    NQC = nq // qc

    from contextlib import ExitStack

    with ExitStack() as ctx:
        cpool = ctx.enter_context(tc.tile_pool(name="const", bufs=1))
        big = ctx.enter_context(tc.tile_pool(name="big", bufs=1))
        vtp = ctx.enter_context(tc.tile_pool(name="vtp", bufs=2))
        gsp = ctx.enter_context(tc.tile_pool(name="gsp", bufs=2))
        epp = ctx.enter_context(tc.tile_pool(name="ep", bufs=2))
        # PSUM: p12 2x1 + stp 2x2 + accp 1x2 = 8 banks
        p12 = ctx.enter_context(tc.tile_pool(name="p12", bufs=2, space="PSUM"))
        stp = ctx.enter_context(tc.tile_pool(name="stps", bufs=2, space="PSUM"))
        accp = ctx.enter_context(tc.tile_pool(name="accps", bufs=1, space="PSUM"))

        ones_sq = cpool.tile([P, P], bf16)
        nc.vector.memset(ones_sq[:], 1.0)

        # ---- HAM warmup: dummy matmuls into the out-accumulator PSUM bank.
        # They only depend on the memset, so they run during the ~9us DMA
        # startup latency; the first real O matmul has start=True and
        # overwrites. Keeps the PE clock-gate at 2.4GHz for the real work.
        warm_rhs = cpool.tile([P, FB], bf16)
        nc.vector.memset(warm_rhs[:], 1.0)
        warm_ps = accp.tile([P, qc], f32, tag="out", name="warm")
        for _ in range(WARM_MM):
            nc.tensor.matmul(warm_ps[:, 0:FB], ones_sq[:], warm_rhs[:],
                             start=True, stop=True)

        w_sb = {}

        def load_w(name):
            t = cpool.tile([P, DT * H], bf16, tag=name, name=f"w_{name}")
            nc.sync.dma_start(
                t[:].rearrange("p (t h) -> p t h", t=DT), Ws[name])
            w_sb[name] = t

        # X^T: xt[p, c, t, nb] = X^T[t*128+p, c*1024+nb] (DMA-chunk major)
        xt = big.tile([P, XC * DT * XCR], bf16)
        xt4 = xt[:].rearrange("p (c t nb) -> p c t nb", c=XC, t=DT)

        def xt_job(hc, t):
            """[128, 512] X^T slice for projection job hc, d-tile t."""
            c = hc * CR // XCR
            o = (hc * CR) % XCR
            return xt4[:, c, t, o:o + CR]
        kT = big.tile([P, n], bf16)          # K^T[h, keys]
        qT = big.tile([P, nq], bf16)         # Q^T[h, q]
        v_sb = big.tile([P, NT * H], bf16)   # V[k%128, kt*H + h]
        v_sb3 = v_sb[:].rearrange("p (kt h) -> p kt h", h=H)
        # exp ring: pT3[:, r, :] = P^T slice for k-tile with kt % PR == r
        pT_all = big.tile([P, PR * qc], bf16)
        pT3 = pT_all[:].rearrange("p (r q) -> p r q", r=PR)

        def produce_data(c):
            nc.sync.dma_start(xt4[:, c], XT[c])

        def produce_slice(c, h2):
            """4-d-tile (512KB) half of chunk c: fine-grained arrival
            while keeping 4KB per-partition DMA lines (2KB lines halve
            the early DMA bandwidth)."""
            nc.sync.dma_start(xt4[:, c, 4 * h2:4 * h2 + 4],
                              XT[c][:, 4 * h2:4 * h2 + 4])

        def proj_pair_stages(jobs, on_scalar=False):
            """Return 4 closures, each emitting 2 t-steps of the pair's
            interleaved matmuls; the last also emits copies/transposes."""
            state = {}

            def stage(si):
                def run():
                    if si == 0:
                        state['tiles'] = [
                            p12.tile([P, CR], f32, tag="pps",
                                     name=f"ps_{w}{c}")
                            for w, c in jobs]
                    for t in range(si * 2, si * 2 + 2):
                        for (wname, c), ps in zip(jobs, state['tiles']):
                            nc.tensor.matmul(
                                ps[:],
                                w_sb[wname][:, t * H:(t + 1) * H],
                                xt_job(c, t),
                                start=(t == 0),
                                stop=(t == DT - 1),
                            )
                    if si == 3:
                        for ji, ((wname, c), ps) in enumerate(
                                zip(jobs, state['tiles'])):
                            # split the pair's two PSUM-evacuation copies
                            # across ACT (job 0) and DVE (job 1) so neither
                            # engine eats both and the p12 slots free fast
                            cp = (nc.scalar.copy if ji == 0
                                  else nc.vector.tensor_copy)
                            if wname == "wv":
                                vt = vtp.tile([P, CR], bf16, tag="vt",
                                              name=f"vt{c}")
                                cp(vt[:], ps[:])
                                nc.sync.dma_start_transpose(
                                    v_sb3[:, c * KPC:(c + 1) * KPC], vt[:])
                            else:
                                dst = kT if wname == "wk" else qT
                                cp(dst[:, c * CR:(c + 1) * CR], ps[:])
                return run
            return [stage(i) for i in range(4)]

        def proj_pair(jobs, on_scalar=False):
            for s in proj_pair_stages(jobs, on_scalar):
                s()

        # ---- Phase 1: weights first, then X chunks fine-grained/early.
        # All DMA triggers are issued up front in need-order; the DGE
        # queues execute them FIFO so later chunks never delay earlier
        # ones, and the first projection matmul only waits for
        # wk + wv + one 2-t-slice of chunk 0 (~0.75MB).
        # chunk-0 first half leads (4KB lines race ahead of the 2KB-line
        # weight DMAs), then weights, then the rest in consumption order.
        produce_slice(0, 0)
        load_w("wk")
        load_w("wv")
        produce_slice(0, 1)
        load_w("wq")
        produce_data(1)
        for c in range(2, XC):
            produce_data(c)
        proj_pair((("wk", 0), ("wv", 0)), on_scalar=True)
        proj_pair((("wq", 0), ("wq", 1)), on_scalar=True)

        def emit_S(q0, kt):
            st = stp.tile([P, qc], f32, tag="st", name=f"st{q0}_{kt}")
            for j in range(0, qc, FB):
                nc.tensor.matmul(
                    st[:, j:j + FB],
                    kT[:, kt * P:(kt + 1) * P],
                    qT[:, q0 + j:q0 + j + FB],
                    start=True, stop=True,
                )
            return st

        # deferred epilogue state from the previous q-chunk
        pending = {}

        def defer_tree_a():
            # acc2 = acc4[0]+acc4[1] | acc4[2]+acc4[3]  (interleaved pairs)
            if not pending:
                return
            a4 = pending['acc4']
            acc2 = epp.tile([P, 2 * qc], bf16, tag="acc2", bufs=1,
                            name="acc2")
            nc.vector.tensor_add(acc2[:], a4[:, 0:2 * qc], a4[:, 2 * qc:])
            pending['acc2'] = acc2

        def defer_tree_b():
            if not pending:
                return
            acc2 = pending.pop('acc2')
            accf = epp.tile([P, qc], bf16, tag="accf", bufs=1, name="accf")
            nc.vector.tensor_add(accf[:], acc2[:, 0:qc], acc2[:, qc:])
            pending['accf'] = accf

        def finish_epilogue():
            if not pending:
                return
            accf, ob, q0p = pending.pop('accf'), pending.pop('ob'), \
                pending.pop('q0')
            pending.pop('acc4')
            l_a = p12.tile([P, FB], f32, tag="pps", name=f"la{q0p}")
            l_b = p12.tile([P, FB], f32, tag="pps", name=f"lb{q0p}")
            r_sb = epp.tile([P, qc], f32, tag="rsb", name=f"rsb{q0p}")
            o_sb = epp.tile([P, qc], f32, tag="osb", name=f"osb{q0p}")
            nc.tensor.matmul(l_a[:], ones_sq[:], accf[:, 0:FB],
                             start=True, stop=True)
            nc.vector.reciprocal_approx_fast(r_sb[:, 0:FB], l_a[:])
            nc.tensor.matmul(l_b[:], ones_sq[:], accf[:, FB:qc],
                             start=True, stop=True)
            nc.gpsimd.tensor_mul(o_sb[:, 0:FB], ob[:, 0:FB], r_sb[:, 0:FB])
            nc.sync.dma_start(OT[:, q0p:q0p + FB], o_sb[:, 0:FB])
            nc.vector.reciprocal_approx_fast(r_sb[:, FB:qc], l_b[:])
            nc.vector.tensor_mul(o_sb[:, FB:qc], ob[:, FB:qc], r_sb[:, FB:qc])
            nc.sync.dma_start(OT[:, q0p + FB:q0p + qc], o_sb[:, FB:qc])

        for qi in range(NQC):
            q0 = qi * qc
            final = (qi == NQC - 1)
            actions = {}
            if qi == 0:
                pjobs = [(("wk", c), ("wv", c)) for c in range(1, NC)]
                pjobs.append((("wq", 2), ("wq", 3)))
                # K1/V1 compressed into the first two slots (needed by S(4))
                # All in-loop proj copies run on ACT (exp leaves ~700ns/kt
                # of ACT slack in qi0) so the DVE never gates the p12
                # PSUM rotation from behind a 2.3us denominator add.
                s10, s11, s12, s13 = proj_pair_stages(pjobs[0],
                                                      on_scalar=True)
                actions.setdefault(0, []).extend([(s10, ()), (s11, ())])
                actions.setdefault(1, []).extend([(s12, ()), (s13, ())])
                at = 2
                for jobs in pjobs[1:]:
                    for s in proj_pair_stages(jobs, on_scalar=True):
                        actions.setdefault(at, []).append((s, ()))
                        at += 1
            else:
                actions.setdefault(1, []).append((defer_tree_a, ()))
                actions.setdefault(2, []).append((defer_tree_b, ()))
                actions.setdefault(5, []).append((finish_epilogue, ()))

            out_ps = accp.tile([P, qc], f32, tag="out", name=f"out{qi}")
            st_tiles = {0: emit_S(q0, 0)}
            # denominator accumulator: [p, 4, qc] bf16, four interleaved
            # partial sums tree-reduced on DVE before the epilogue
            # ones-matmul (partition reduction only).
            acc4 = gsp.tile([P, GL * qc], bf16, tag="a4", name=f"a4_{qi}")
            fin = {}  # final-chunk epilogue tiles
            for kt in range(NT):
                # S(kt+1) is emitted FIRST so it can never queue behind a
                # stalled projection matmul in the PE FIFO (the proj
                # pipeline is gated on copies; S leading breaks the
                # proj->copy->exp->S convoy cycle).
                if kt + 1 < NT:
                    st_tiles[kt + 1] = emit_S(q0, kt + 1)
                # exp on ACT into the ring
                nc.scalar.activation(
                    pT3[:, kt % PR, :], st_tiles.pop(kt)[:],
                    AF.Exp, scale=SCALE)
                # O^T accumulation for the PREVIOUS kt (software pipeline)
                if kt > 0:
                    for j in range(0, qc, FB):
                        nc.tensor.matmul(
                            out_ps[:, j:j + FB],
                            v_sb3[:, kt - 1, :],
                            pT3[:, (kt - 1) % PR, j:j + FB],
                            start=(kt - 1 == 0), stop=False,
                        )
                for fn, arg in actions.get(kt, ()):
                    fn(*arg)
                # denominator: one contiguous 4-slice DVE add per 4 k-tiles.
                # The final q-chunk keeps its last 8 k-tiles out of the
                # accumulator and spreads its tree reduction across kts
                # 25-31 (each step gated only on an already-finished exp)
                # so no multi-us DVE convoy forms at the very end.
                ngrp = NT - 2 * GL if final else NT
                if kt < ngrp and kt % GL == GL - 1:
                    r0 = (kt - (GL - 1)) % PR
                    grp = pT_all[:, r0 * qc:(r0 + GL) * qc]
                    if kt == GL - 1:
                        nc.vector.tensor_copy(acc4[:], grp)
                    else:
                        nc.vector.tensor_add(acc4[:], acc4[:], grp)
                if final:
                    if kt == 25:
                        a2 = epp.tile([P, 2 * qc], bf16, tag="acc2f",
                                      bufs=1, name="acc2f")
                        nc.vector.tensor_add(
                            a2[:], acc4[:, 0:2 * qc], acc4[:, 2 * qc:])
                        fin['a2'] = a2
                    elif kt == 26:
                        af = epp.tile([P, qc], bf16, tag="accff", bufs=1,
                                      name="accff")
                        nc.vector.tensor_add(
                            af[:], fin['a2'][:, 0:qc], fin['a2'][:, qc:])
                        fin['af'] = af
                    elif kt in (27, 29, 31):
                        # pair-sums of raw ring slices (24,25),(26,27),(28,29)
                        k0 = kt - 3 if kt != 31 else 28
                        pt = epp.tile([P, qc], bf16, tag=f"pp{k0}",
                                      bufs=1, name=f"pp{k0}")
                        nc.vector.tensor_add(
                            pt[:], pT3[:, k0 % PR, :],
                            pT3[:, (k0 + 1) % PR, :])
                        fin[f'p{k0}'] = pt

            if not final:
                # last O^T tile, evacuate on DVE (ACT stays exp-only),
                # defer the l/recip/scale chain into the next q-chunk.
                for j in range(0, qc, FB):
                    nc.tensor.matmul(
                        out_ps[:, j:j + FB],
                        v_sb3[:, NT - 1, :],
                        pT3[:, (NT - 1) % PR, j:j + FB],
                        start=False, stop=True,
                    )
                ob = epp.tile([P, qc], f32, tag="ob", name=f"ob{qi}")
                nc.vector.tensor_copy(ob[:], out_ps[:])
                pending.update(acc4=acc4, ob=ob, q0=q0)
            else:
                # ---- final-chunk tail: everything that can run before
                # exp(31) is emitted first; the post-exp(31) chain is
                # [O31-half, l-last-matmul] x2 -> recip (on ACT, free by
                # then) -> DVE mul -> DMA, quartered so scale and
                # DMA-out overlap.
                l_a = p12.tile([P, FB], f32, tag="pps", name="la_f")
                l_b = p12.tile([P, FB], f32, tag="pps", name="lb_f")
                terms = [fin['af'], fin['p24'], fin['p26'], fin['p28'],
                         pT3[:, (NT - 2) % PR, :]]
                for g, t in enumerate(terms):
                    nc.tensor.matmul(l_a[:], ones_sq[:], t[:, 0:FB],
                                     start=(g == 0), stop=False)
                for g, t in enumerate(terms):
                    nc.tensor.matmul(l_b[:], ones_sq[:], t[:, FB:qc],
                                     start=(g == 0), stop=False)
                last = pT3[:, (NT - 1) % PR, :]
                r_sb = epp.tile([P, qc], f32, tag="rsb", name="rsb_f")
                o_sb = epp.tile([P, qc], f32, tag="osb", name="osb_f")
                for j, l in ((0, l_a), (FB, l_b)):
                    nc.tensor.matmul(
                        out_ps[:, j:j + FB],
                        v_sb3[:, NT - 1, :],
                        pT3[:, (NT - 1) % PR, j:j + FB],
                        start=False, stop=True,
                    )
                    nc.tensor.matmul(l[:], ones_sq[:], last[:, j:j + FB],
                                     start=False, stop=True)
                    nc.vector.reciprocal_approx_fast(r_sb[:, j:j + FB],
                                                     l[:])
                    for jq in (j, j + FB // 2):
                        nc.vector.tensor_mul(
                            o_sb[:, jq:jq + FB // 2],
                            out_ps[:, jq:jq + FB // 2],
                            r_sb[:, jq:jq + FB // 2])
                        nc.sync.dma_start(OT[:, q0 + jq:q0 + jq + FB // 2],
                                          o_sb[:, jq:jq + FB // 2])

        finish_epilogue()


def build_bass(n=N, d=D, nq=NQ):
    import concourse.mybir as mybir
    from concourse import bacc
    from concourse.tile import TileContext

    dt = mybir.dt
    nc = bacc.Bacc("TRN2", target_bir_lowering=False, debug=False)
    XT = nc.dram_tensor(
        "XT", [XC, P, DT, XCR], dt.bfloat16, kind="ExternalInput").ap()
    Ws = {}
    for name in ("wq", "wk", "wv"):
        Ws[name] = nc.dram_tensor(
            name.upper(), [P, DT, H], dt.bfloat16, kind="ExternalInput").ap()
    OT = nc.dram_tensor("OT", [H, nq], dt.float32, kind="ExternalOutput").ap()

    with TileContext(nc) as tc:
        emit_attention(tc, XT, Ws, OT, n=n, d=d, nq=nq)
    nc.compile()  # bacc passes: split multi-waits into EVSEM chains, etc.
    return nc


_CACHED = {}


def _get_nc():
    if "nc" not in _CACHED:
        _CACHED["nc"] = build_bass()
    return _CACHED["nc"]


def _prep_w(w):
    import ml_dtypes
    # [D, H] f32 -> [128, DT, H] bf16 with w_t[p, t, h] = W[t*128+p, h]
    return np.ascontiguousarray(
        w.reshape(DT, P, H).transpose(1, 0, 2)).astype(ml_dtypes.bfloat16)


def _prep_xt(xb):
    import ml_dtypes
    # [N, D] f32 -> [XC, 128, DT, XCR] bf16:
    # XT[c, p, t, nb] = X[c*XCR+nb, t*128+p]
    x4 = xb.reshape(XC, XCR, DT, P)          # [c, nb, t, p]
    return np.ascontiguousarray(
        x4.transpose(0, 3, 2, 1)).astype(ml_dtypes.bfloat16)


def kernel(X, Wq, Wk, Wv, trace=False):
    """Full-input entry point: X [4,4096,1024] f32 -> [4,4096,128] f32."""
    from concourse.bass_utils import run_bass_kernel_spmd

    X = np.ascontiguousarray(X, dtype=np.float32)
    wmap = {"WQ": _prep_w(np.asarray(Wq, dtype=np.float32)),
            "WK": _prep_w(np.asarray(Wk, dtype=np.float32)),
            "WV": _prep_w(np.asarray(Wv, dtype=np.float32))}

    nc = _get_nc()
    in_maps = []
    for core in range(NCORES):
        b, half = core // QSPLIT, core % QSPLIT
        xb = X[b]
        if half:
            # roll so this core's queries are rows [0:NQ); key set is unchanged
            xb = np.concatenate([xb[NQ:], xb[:NQ]], axis=0)
        in_maps.append({"XT": _prep_xt(xb), **wmap})

    res = run_bass_kernel_spmd(
        nc, in_maps, core_ids=list(range(NCORES)), trace=trace
    )
    out = np.empty((B, N, H), dtype=np.float32)
    for core in range(NCORES):
        b, half = core // QSPLIT, core % QSPLIT
        out[b, half * NQ:(half + 1) * NQ] = res.results[core]["OT"].T
    if trace:
        return out, res
    return out
